# revision 1
# baseline (speedup 1.0000x reference)
"""Trainium2 Bass kernel for nn_EventTemplateBank (batched 1-D template-bank conv).

Math: score[b,t,e] = sum_{f,l} delayed[e,f,l] * x[b, t+40-l, f] / (L*F),
with delayed = delay-shifted templates (zero fill) and x zero-padded.

Device formulation (per core, data-parallel over batch):
  - Contract over a 128-position window on SBUF partitions.
  - Host pre-permutes x into overlapping-window scratch with one flat
    column axis across the core's 8 batches (683 columns per batch,
    zero-padded to 11*512):
        Xsc[k, f, c] = x[b, 48n + k - 39, f],  c = 683*b + n
    so every output t = 48n + D (D in [0,48)) has its full 80-tap window
    inside the k range of column c.
  - Toeplitz weights (host-built from the tiny templates):
        W[k, s, f, 16d+e] = delayed[e, f, (8s+d) + 79 - k] / 480
    One PSUM tile per D-set s accumulates 6 matmuls (one per feature f):
        out[s][m=(d,e), c-block] += W[:, s, f].T @ Xsc[:, f, c-block]
    Operands are float32r (single-pass PE, ~1 cycle/column at N=512).
  - Output written to DRAM in matmul-native layout; host re-permutes to (B,S,E).
"""

import numpy as np

import concourse.mybir as mybir
from concourse import bacc
from concourse.bass_utils import run_bass_kernel_spmd
from concourse.tile import TileContext

# Problem shapes (hardcoded per contract)
B, S, F = 64, 32768, 6
E, L = 16, 80
MAX_DELAY = 10

NCORES = 8
BPC = B // NCORES          # batches per core
Q = 48                     # output positions per rhs column
KWIN = 128                 # contraction window (partitions)
NS = 6                     # D-sets of 8 -> D in [0, 48)
PADF = 39                  # window of column n starts at 48n - 39
NCOLB = (S + Q - 1) // Q   # 683 columns per batch
BLKN = 512                 # columns per matmul block
NBLK = 11                  # ceil(8*683 / 512)
CPAD = NBLK * BLKN         # 5632 padded columns per core
CTOT = BPC * NCOLB         # 5464 real columns per core
LASTN = CTOT - (NBLK - 1) * BLKN   # 344 real columns in the last block

LAST_RESULT = None         # BassKernelResults of the most recent run (for profiling)


def _build_weights(templates: np.ndarray, onset_delays: np.ndarray) -> np.ndarray:
    """W[k, s, f, 16d+e] = delayed[e, f, (8s+d)+79-k] / (L*F), zero outside [0,L)."""
    d = np.round(np.clip(onset_delays, -MAX_DELAY, MAX_DELAY)).astype(np.int64)
    idx = np.arange(L)
    src = idx[None, None, :] - d[:, :, None]                 # (E,F,L)
    valid = (src >= 0) & (src < L)
    delayed = np.take_along_axis(templates, np.clip(src, 0, L - 1), axis=2)
    delayed = np.where(valid, delayed, 0.0).astype(np.float32) / float(L * F)

    D = (8 * np.arange(NS)[:, None] + np.arange(8)[None, :])      # (NS, 8)
    l_idx = D[:, :, None] + 79 - np.arange(KWIN)[None, None, :]   # (NS, 8, K)
    ok = (l_idx >= 0) & (l_idx < L)
    g = delayed[:, :, np.clip(l_idx, 0, L - 1)]                   # (E, F, NS, 8, K)
    g = np.where(ok[None, None], g, 0.0)
    # -> W[k, s, f, dd, e] (k-major so the device DMA is contiguous)
    W = g.transpose(4, 2, 1, 3, 0).reshape(KWIN, NS, F, 128)
    return np.ascontiguousarray(W, dtype=np.float32)


def _build_xsc(x: np.ndarray) -> np.ndarray:
    """Xsc[core, k, f, c] = x[8*core + c//683, 48*(c%683) + k - 39, f], zero OOB/pad."""
    need = Q * (NCOLB - 1) + KWIN
    xpad = np.zeros((B, PADF + need, F), dtype=np.float32)
    xpad[:, PADF:PADF + S, :] = x
    sb, st, sf = xpad.strides
    v = np.lib.stride_tricks.as_strided(
        xpad, shape=(B, KWIN, F, NCOLB), strides=(sb, st, sf, Q * st)
    )
    out = np.zeros((NCORES, KWIN, F, CPAD), dtype=np.float32)
    for b in range(B):
        core, i = divmod(b, BPC)
        out[core, :, :, i * NCOLB:(i + 1) * NCOLB] = v[b]
    return out


def _build_program():
    f32 = mybir.dt.float32
    f32r = mybir.dt.float32r
    nc = bacc.Bacc("TRN2", target_bir_lowering=False, debug=False)
    xsc = nc.dram_tensor("xsc", [KWIN, F, CPAD], f32, kind="ExternalInput")
    w = nc.dram_tensor("w", [KWIN, NS, F, 128], f32, kind="ExternalInput")
    osc = nc.dram_tensor("osc", [NBLK, NS, 128, BLKN], f32, kind="ExternalOutput")

    with TileContext(nc) as tc:
        with (
            tc.tile_pool(name="wp", bufs=1) as wp,
            tc.tile_pool(name="xp", bufs=20) as xp,
            tc.tile_pool(name="pp", bufs=8, space="PSUM") as pp,
            tc.tile_pool(name="op", bufs=6) as op,
        ):
            # Weights: per-set DMA + DVE cast-copy to float32r.
            wt_raw = wp.tile([KWIN, NS * F * 128], f32)
            wt = wp.tile([KWIN, NS * F * 128], f32r)
            wr = w.rearrange("k s f m -> k (s f m)")
            for s in range(NS):
                if s == 0:
                    # s=0 split per feature: the first matmul gates on a 64KB
                    # piece + 130ns cast instead of the whole 384KB set.
                    for f in range(F):
                        sl = slice(f * 128, (f + 1) * 128)
                        nc.sync.dma_start(out=wt_raw[:, sl], in_=wr[:, sl])
                        nc.vector.tensor_copy(out=wt[:, sl], in_=wt_raw[:, sl])
                else:
                    sl = slice(s * F * 128, (s + 1) * F * 128)
                    nc.sync.dma_start(out=wt_raw[:, sl], in_=wr[:, sl])
                    nc.vector.tensor_copy(out=wt[:, sl], in_=wt_raw[:, sl])
            for blk in range(NBLK):
                n = BLKN if blk < NBLK - 1 else LASTN
                # One SWDGE cast-DMA (f32 DRAM -> f32r SBUF) per feature plane:
                # matmuls gate on single 256KB planes, not the whole 1.5MB block.
                xtp = []
                for f in range(F):
                    xf = xp.tile([KWIN, n], f32r, tag="xtp")
                    nc.gpsimd.dma_start(
                        out=xf, in_=xsc[:, f, blk * BLKN:blk * BLKN + n]
                    )
                    xtp.append(xf)
                pss = [
                    pp.tile([128, n], f32, tag="ps", name=f"ps_{blk}_{s}")
                    for s in range(NS)
                ]

                def evac(s, n=n, blk=blk, pss=pss):
                    ot = op.tile([128, n], f32, tag="ot", name=f"ot_{blk}_{s}")
                    nc.vector.tensor_copy(out=ot, in_=pss[s])
                    nc.sync.dma_start(out=osc[blk, s, :, 0:n], in_=ot)

                if blk == 0:
                    # f-outer: each arriving x-plane feeds all 6 sets, so the
                    # PE starts as soon as the first 256KB plane lands.
                    for f in range(F):
                        for s in range(NS):
                            nc.tensor.matmul(
                                pss[s],
                                wt[:, (s * F + f) * 128:(s * F + f + 1) * 128],
                                xtp[f],
                                start=(f == 0),
                                stop=(f == F - 1),
                                skip_group_check=True,
                            )
                    for s in range(NS):
                        evac(s)
                else:
                    # s-outer: sets complete one after another, so PSUM
                    # evacuation + output DMA stagger across the block.
                    for s in range(NS):
                        for f in range(F):
                            nc.tensor.matmul(
                                pss[s],
                                wt[:, (s * F + f) * 128:(s * F + f + 1) * 128],
                                xtp[f],
                                start=(f == 0),
                                stop=(f == F - 1),
                            )
                        evac(s)
    nc.compile()   # bacc passes: split multi-waits (HW allows 1 wait/inst), DCE, reg alloc
    return nc


def kernel(x: np.ndarray, templates: np.ndarray, onset_delays: np.ndarray) -> np.ndarray:
    global LAST_RESULT
    x = np.ascontiguousarray(x, dtype=np.float32)
    templates = np.asarray(templates, dtype=np.float32)
    onset_delays = np.asarray(onset_delays, dtype=np.float32)

    W = _build_weights(templates, onset_delays)
    Xsc = _build_xsc(x)                                   # (NCORES, K, F, CPAD)

    nc = _build_program()
    in_maps = [{"xsc": Xsc[c], "w": W} for c in range(NCORES)]
    res = run_bass_kernel_spmd(nc, in_maps, core_ids=list(range(NCORES)))
    LAST_RESULT = res

    osc = np.stack([r["osc"] for r in res.results], axis=0)   # (NCORES,NBLK,NS,128,BLKN)
    o = osc.reshape(NCORES, NBLK, NS, 8, E, BLKN)             # core, blk, s, d, e, n
    o = o.transpose(0, 1, 5, 2, 3, 4)                          # core, blk, n, s, d, e
    o = np.ascontiguousarray(o).reshape(NCORES, CPAD, NS * 8 * E)
    o = o[:, :BPC * NCOLB, :].reshape(NCORES, BPC, NCOLB, NS, 8, E)
    o = o.reshape(B, NCOLB * Q, E)[:, :S, :]
    o = np.ascontiguousarray(o)
    o[:, S - 1, :] = 0.0                                   # reference zero-pads last column
    return o



# revision 2
# speedup vs baseline: 1.0450x; 1.0450x over previous
"""Trainium2 Bass kernel for nn_EventTemplateBank (batched 1-D template-bank conv).

Math: score[b,t,e] = sum_{f,l} delayed[e,f,l] * x[b, t+40-l, f] / (L*F),
with delayed = delay-shifted templates (zero fill) and x zero-padded.

Device formulation (per core, data-parallel over batch):
  - Contract over a 128-position window on SBUF partitions.
  - Host pre-permutes x into overlapping-window scratch (bf16) with one flat
    column axis across the core's 8 batches (683 columns per batch,
    zero-padded to 11*512), stored block-major so each block is one DMA:
        Xsc[blk, k, f, j] = x[b, 48n + k - 39, f],  c = 512*blk + j = 683*b + n
    so every output t = 48n + D (D in [0,48)) has its full 80-tap window
    inside the k range of column c.
  - Toeplitz weights (host-built bf16 from the tiny templates):
        W[k, s, f, 16d+e] = delayed[e, f, (8s+d) + 79 - k] / 480
    One PSUM tile per D-set s accumulates 6 matmuls (one per feature f):
        out[s][m=(d,e), c-block] += W[:, s, f].T @ Xsc[:, f, c-block]
    bf16 operands stream 1 column/cycle through the 128x128 PE.
  - PSUM f32 evacuated as bf16; output DMA'd in matmul-native layout;
    host upcasts to f32 and re-permutes to (B,S,E).
"""

import numpy as np
import ml_dtypes

import concourse.mybir as mybir
from concourse import bacc
from concourse.bass_utils import run_bass_kernel_spmd
from concourse.tile import TileContext

BF16 = ml_dtypes.bfloat16

# Problem shapes (hardcoded per contract)
B, S, F = 64, 32768, 6
E, L = 16, 80
MAX_DELAY = 10

NCORES = 8
BPC = B // NCORES          # batches per core
Q = 48                     # output positions per rhs column
KWIN = 128                 # contraction window (partitions)
NS = 6                     # D-sets of 8 -> D in [0, 48)
PADF = 39                  # window of column n starts at 48n - 39
NCOLB = (S + Q - 1) // Q   # 683 columns per batch
BLKN = 512                 # columns per matmul block
NBLK = 11                  # ceil(8*683 / 512)
CPAD = NBLK * BLKN         # 5632 padded columns per core
CTOT = BPC * NCOLB         # 5464 real columns per core
LASTN = CTOT - (NBLK - 1) * BLKN   # 344 real columns in the last block

LAST_RESULT = None         # BassKernelResults of the most recent run (for profiling)


def _build_weights(templates: np.ndarray, onset_delays: np.ndarray) -> np.ndarray:
    """W[k, s, f, 16d+e] = delayed[e, f, (8s+d)+79-k] / (L*F), zero outside [0,L)."""
    d = np.round(np.clip(onset_delays, -MAX_DELAY, MAX_DELAY)).astype(np.int64)
    idx = np.arange(L)
    src = idx[None, None, :] - d[:, :, None]                 # (E,F,L)
    valid = (src >= 0) & (src < L)
    delayed = np.take_along_axis(templates, np.clip(src, 0, L - 1), axis=2)
    delayed = np.where(valid, delayed, 0.0).astype(np.float32) / float(L * F)

    D = (8 * np.arange(NS)[:, None] + np.arange(8)[None, :])      # (NS, 8)
    l_idx = D[:, :, None] + 79 - np.arange(KWIN)[None, None, :]   # (NS, 8, K)
    ok = (l_idx >= 0) & (l_idx < L)
    g = delayed[:, :, np.clip(l_idx, 0, L - 1)]                   # (E, F, NS, 8, K)
    g = np.where(ok[None, None], g, 0.0)
    # -> W[k, s, f, dd, e] (k-major so the device DMA is contiguous)
    W = g.transpose(4, 2, 1, 3, 0).reshape(KWIN, NS, F, 128)
    return np.ascontiguousarray(W).astype(BF16)


def _build_xsc(x: np.ndarray) -> np.ndarray:
    """Xsc[core, blk, k, f, j] = x[8*core + c//683, 48*(c%683) + k - 39, f],
    c = 512*blk + j, zero OOB/pad. bf16."""
    need = Q * (NCOLB - 1) + KWIN
    xpad = np.zeros((B, PADF + need, F), dtype=np.float32)
    xpad[:, PADF:PADF + S, :] = x
    sb, st, sf = xpad.strides
    v = np.lib.stride_tricks.as_strided(
        xpad, shape=(B, KWIN, F, NCOLB), strides=(sb, st, sf, Q * st)
    )
    out = np.zeros((NCORES, KWIN, F, CPAD), dtype=BF16)
    v16 = v.astype(BF16)
    for b in range(B):
        core, i = divmod(b, BPC)
        out[core, :, :, i * NCOLB:(i + 1) * NCOLB] = v16[b]
    # block-major: (core, NBLK, KWIN, F, BLKN) so one DMA per block moves
    # 6 KB contiguous per partition.
    out = out.reshape(NCORES, KWIN, F, NBLK, BLKN).transpose(0, 3, 1, 2, 4)
    return np.ascontiguousarray(out)


def _build_program():
    f32 = mybir.dt.float32
    bf16 = mybir.dt.bfloat16
    nc = bacc.Bacc("TRN2", target_bir_lowering=False, debug=False)
    xsc = nc.dram_tensor("xsc", [NBLK, KWIN, F, BLKN], bf16, kind="ExternalInput")
    w = nc.dram_tensor("w", [KWIN, NS, F, 128], bf16, kind="ExternalInput")
    osc = nc.dram_tensor("osc", [NBLK, NS, 128, BLKN], bf16, kind="ExternalOutput")

    with TileContext(nc) as tc:
        with (
            tc.tile_pool(name="wp", bufs=1) as wp,
            tc.tile_pool(name="xp", bufs=4) as xp,
            tc.tile_pool(name="pp", bufs=8, space="PSUM") as pp,
            tc.tile_pool(name="op", bufs=6) as op,
        ):
            # Weights: straight bf16 DMA, split so the first matmuls gate on
            # small pieces.
            wt = wp.tile([KWIN, NS * F * 128], bf16)
            wr = w.rearrange("k s f m -> k (s f m)")
            for f in range(F):
                sl = slice(f * 128, (f + 1) * 128)
                nc.sync.dma_start(out=wt[:, sl], in_=wr[:, sl])
            nc.sync.dma_start(out=wt[:, F * 128:], in_=wr[:, F * 128:])
            xr = xsc.rearrange("b k f c -> b k (f c)")
            for blk in range(NBLK):
                n = BLKN if blk < NBLK - 1 else LASTN
                # one DMA per block: [128, 6*512] bf16, 6KB per partition
                xt = xp.tile([KWIN, F * BLKN], bf16, tag="xt")
                nc.sync.dma_start(out=xt, in_=xr[blk])
                pss = [
                    pp.tile([128, n], f32, tag="ps", name=f"ps_{blk}_{s}")
                    for s in range(NS)
                ]

                def evac(s, n=n, blk=blk, pss=pss):
                    ot = op.tile([128, n], bf16, tag="ot", name=f"ot_{blk}_{s}")
                    nc.vector.tensor_copy(out=ot, in_=pss[s])
                    nc.sync.dma_start(out=osc[blk, s, :, 0:n], in_=ot)

                if blk == 0:
                    # f-outer: each feature's weight piece gates only its own
                    # 6 matmuls, so the PE starts right after the first 32KB
                    # of weights + the first x block land.
                    for f in range(F):
                        for s in range(NS):
                            nc.tensor.matmul(
                                pss[s],
                                wt[:, (s * F + f) * 128:(s * F + f + 1) * 128],
                                xt[:, f * BLKN:f * BLKN + n],
                                start=(f == 0),
                                stop=(f == F - 1),
                                skip_group_check=True,
                            )
                    for s in range(NS):
                        evac(s)
                else:
                    # s-outer: sets complete one after another, so PSUM
                    # evacuation + output DMA stagger across the block.
                    for s in range(NS):
                        for f in range(F):
                            nc.tensor.matmul(
                                pss[s],
                                wt[:, (s * F + f) * 128:(s * F + f + 1) * 128],
                                xt[:, f * BLKN:f * BLKN + n],
                                start=(f == 0),
                                stop=(f == F - 1),
                            )
                        evac(s)
    nc.compile()   # bacc passes: split multi-waits (HW allows 1 wait/inst), DCE, reg alloc
    return nc


def kernel(x: np.ndarray, templates: np.ndarray, onset_delays: np.ndarray) -> np.ndarray:
    global LAST_RESULT
    x = np.ascontiguousarray(x, dtype=np.float32)
    templates = np.asarray(templates, dtype=np.float32)
    onset_delays = np.asarray(onset_delays, dtype=np.float32)

    W = _build_weights(templates, onset_delays)
    Xsc = _build_xsc(x)                                   # (NCORES, NBLK, K, F, BLKN)

    nc = _build_program()
    in_maps = [{"xsc": Xsc[c], "w": W} for c in range(NCORES)]
    res = run_bass_kernel_spmd(nc, in_maps, core_ids=list(range(NCORES)))
    LAST_RESULT = res

    osc = np.stack([r["osc"] for r in res.results], axis=0)   # (NCORES,NBLK,NS,128,BLKN)
    osc = osc.astype(np.float32)
    o = osc.reshape(NCORES, NBLK, NS, 8, E, BLKN)             # core, blk, s, d, e, n
    o = o.transpose(0, 1, 5, 2, 3, 4)                          # core, blk, n, s, d, e
    o = np.ascontiguousarray(o).reshape(NCORES, CPAD, NS * 8 * E)
    o = o[:, :BPC * NCOLB, :].reshape(NCORES, BPC, NCOLB, NS, 8, E)
    o = o.reshape(B, NCOLB * Q, E)[:, :S, :]
    o = np.ascontiguousarray(o)
    o[:, S - 1, :] = 0.0                                   # reference zero-pads last column
    return o


# revision 7
# speedup vs baseline: 1.0912x; 1.0442x over previous
"""Trainium2 Bass kernel for nn_EventTemplateBank (batched 1-D template-bank conv).

Math: score[b,t,e] = sum_{f,l} delayed[e,f,l] * x[b, t+40-l, f] / (L*F),
with delayed = delay-shifted templates (zero fill) and x zero-padded.

Device formulation (per core, data-parallel over batch):
  - Contract over a 128-position window on SBUF partitions.
  - Host pre-permutes x into overlapping-window scratch (bf16) with one flat
    column axis across the core's 8 batches (683 columns per batch,
    zero-padded to 11*512), stored block-major so each block is one DMA:
        Xsc[blk, k, f, j] = x[b, 48n + k - 39, f],  c = 512*blk + j = 683*b + n
    so every output t = 48n + D (D in [0,48)) has its full 80-tap window
    inside the k range of column c.
  - Toeplitz weights (host-built bf16 from the tiny templates):
        W[k, s, f, 16d+e] = delayed[e, f, (8s+d) + 79 - k] / 480
    One PSUM tile per D-set s accumulates 6 matmuls (one per feature f):
        out[s][m=(d,e), c-block] += W[:, s, f].T @ Xsc[:, f, c-block]
    bf16 operands stream 1 column/cycle through the 128x128 PE.
  - PSUM f32 evacuated as bf16; output DMA'd in matmul-native layout;
    host upcasts to f32 and re-permutes to (B,S,E).
"""

import numpy as np
import ml_dtypes

import concourse.mybir as mybir
from concourse import bacc
from concourse.bass_utils import run_bass_kernel_spmd
from concourse.tile import TileContext

BF16 = ml_dtypes.bfloat16

# Problem shapes (hardcoded per contract)
B, S, F = 64, 32768, 6
E, L = 16, 80
MAX_DELAY = 10

NCORES = 8
BPC = B // NCORES          # batches per core
Q = 48                     # output positions per rhs column
KWIN = 128                 # contraction window (partitions)
NS = 6                     # D-sets of 8 -> D in [0, 48)
PADF = 39                  # window of column n starts at 48n - 39
NCOLB = (S + Q - 1) // Q   # 683 columns per batch
BLKN = 512                 # max columns per matmul block (one PSUM bank)
CTOT = BPC * NCOLB         # 5464 real columns per core
# Small blocks first so the PE starts after ~0.2MB of x instead of 1.5MB
# (and ramps its p-state on cheap blocks); 344-col remainder last so the
# matmul->cast->store drain chain is short.
BLOCKS = [128, 128, 256] + [512] * 9 + [344]
assert sum(BLOCKS) == CTOT
NBLK = len(BLOCKS)
BOFF = [sum(BLOCKS[:i]) for i in range(NBLK)]   # column offset of each block

LAST_RESULT = None         # BassKernelResults of the most recent run (for profiling)


def _build_weights(templates: np.ndarray, onset_delays: np.ndarray) -> np.ndarray:
    """W[k, s, f, 16d+e] = delayed[e, f, (8s+d)+79-k] / (L*F), zero outside [0,L)."""
    d = np.round(np.clip(onset_delays, -MAX_DELAY, MAX_DELAY)).astype(np.int64)
    idx = np.arange(L)
    src = idx[None, None, :] - d[:, :, None]                 # (E,F,L)
    valid = (src >= 0) & (src < L)
    delayed = np.take_along_axis(templates, np.clip(src, 0, L - 1), axis=2)
    delayed = np.where(valid, delayed, 0.0).astype(np.float32) / float(L * F)

    D = (8 * np.arange(NS)[:, None] + np.arange(8)[None, :])      # (NS, 8)
    l_idx = D[:, :, None] + 79 - np.arange(KWIN)[None, None, :]   # (NS, 8, K)
    ok = (l_idx >= 0) & (l_idx < L)
    g = delayed[:, :, np.clip(l_idx, 0, L - 1)]                   # (E, F, NS, 8, K)
    g = np.where(ok[None, None], g, 0.0)
    # -> W[k, f, s, dd, e]: f-major so one contiguous 192KB piece per feature
    # covers all 6 D-sets (the f-outer blocks gate on single features).
    W = g.transpose(4, 1, 2, 3, 0).reshape(KWIN, F, NS, 128)
    return np.ascontiguousarray(W).astype(BF16)


def _build_xsc(x: np.ndarray) -> np.ndarray:
    """Xsc[core, k, :] = concat over blocks of x-windows [F, n_blk]:
    window c = 683*b + n reads x[8*core + b, 48n + k - 39, f]. bf16.
    Block-major rows so each block is one DMA with F*n contiguous bytes
    per partition."""
    need = Q * (NCOLB - 1) + KWIN
    xpad = np.zeros((B, PADF + need, F), dtype=np.float32)
    xpad[:, PADF:PADF + S, :] = x
    sb, st, sf = xpad.strides
    v = np.lib.stride_tricks.as_strided(
        xpad, shape=(B, KWIN, F, NCOLB), strides=(sb, st, sf, Q * st)
    )
    flat = np.empty((NCORES, KWIN, F, CTOT), dtype=BF16)
    v16 = v.astype(BF16)
    for b in range(B):
        core, i = divmod(b, BPC)
        flat[core, :, :, i * NCOLB:(i + 1) * NCOLB] = v16[b]
    out = np.empty((NCORES, KWIN, F * CTOT), dtype=BF16)
    for blk, (off, n) in enumerate(zip(BOFF, BLOCKS)):
        dst = slice(F * off, F * (off + n))
        out[:, :, dst] = flat[:, :, :, off:off + n].reshape(NCORES, KWIN, F * n)
    return np.ascontiguousarray(out)


N_FOUTER = 3               # leading blocks run f-outer (gate on per-f W pieces)


def _build_program():
    f32 = mybir.dt.float32
    bf16 = mybir.dt.bfloat16
    nc = bacc.Bacc("TRN2", target_bir_lowering=False, debug=False)
    xsc = nc.dram_tensor("xsc", [KWIN, F * CTOT], bf16, kind="ExternalInput")
    w = nc.dram_tensor("w", [KWIN, F, NS, 128], bf16, kind="ExternalInput")
    osc = nc.dram_tensor("osc", [NS, 128, CTOT], bf16, kind="ExternalOutput")

    with TileContext(nc) as tc:
        with (
            tc.tile_pool(name="wp", bufs=1) as wp,
            tc.tile_pool(name="xp", bufs=6) as xp,
            tc.tile_pool(name="pp", bufs=8, space="PSUM") as pp,
            tc.tile_pool(name="op", bufs=8) as op,
        ):
            wt = wp.tile([KWIN, F * NS * 128], bf16)     # [k, (f, s, m)]
            wr = w.rearrange("k f s m -> k (f s m)")
            xtiles = {}

            def issue_w(f):
                sl = slice(f * NS * 128, (f + 1) * NS * 128)
                nc.sync.dma_start(out=wt[:, sl], in_=wr[:, sl])

            def issue_x(blk):
                off, n = BOFF[blk], BLOCKS[blk]
                xt = xp.tile([KWIN, F * n], bf16, tag="xt", name=f"xt_{blk}")
                nc.sync.dma_start(out=xt, in_=xsc[:, F * off:F * (off + n)])
                xtiles[blk] = xt

            def wslice(f, s):
                return wt[:, (f * NS + s) * 128:(f * NS + s + 1) * 128]

            # DMA order: the first matmul gates on Wf0+X0 (~0.4MB); later
            # f-pieces and x blocks stream in behind it.
            issue_w(0); issue_x(0)
            issue_w(1); issue_x(1)
            issue_w(2); issue_x(2)
            issue_w(3); issue_w(4); issue_w(5)

            for blk in range(NBLK):
                off, n = BOFF[blk], BLOCKS[blk]
                if blk + 3 < NBLK:
                    issue_x(blk + 3)
                xt = xtiles.pop(blk)
                pss = [
                    pp.tile([128, n], f32, tag="ps", name=f"ps_{blk}_{s}")
                    for s in range(NS)
                ]

                def evac(s, n=n, off=off, blk=blk, pss=pss):
                    ot = op.tile([128, n], bf16, tag="ot", name=f"ot_{blk}_{s}")
                    nc.vector.tensor_copy(out=ot, in_=pss[s])
                    nc.sync.dma_start(out=osc[s, :, off:off + n], in_=ot)

                if blk < N_FOUTER:
                    # f-outer: each arriving f-piece of W feeds all 6 sets,
                    # so the PE starts as soon as Wf0+X0 land.
                    for f in range(F):
                        for s in range(NS):
                            nc.tensor.matmul(
                                pss[s],
                                wslice(f, s),
                                xt[:, f * n:(f + 1) * n],
                                start=(f == 0),
                                stop=(f == F - 1),
                                skip_group_check=True,
                            )
                    for s in range(NS):
                        evac(s)
                else:
                    # s-outer: sets complete one after another, so PSUM
                    # evacuation + output DMA stagger across the block.
                    for s in range(NS):
                        for f in range(F):
                            nc.tensor.matmul(
                                pss[s],
                                wslice(f, s),
                                xt[:, f * n:(f + 1) * n],
                                start=(f == 0),
                                stop=(f == F - 1),
                            )
                        evac(s)
    nc.compile()   # bacc passes: split multi-waits (HW allows 1 wait/inst), DCE, reg alloc
    return nc


def kernel(x: np.ndarray, templates: np.ndarray, onset_delays: np.ndarray) -> np.ndarray:
    global LAST_RESULT
    x = np.ascontiguousarray(x, dtype=np.float32)
    templates = np.asarray(templates, dtype=np.float32)
    onset_delays = np.asarray(onset_delays, dtype=np.float32)

    W = _build_weights(templates, onset_delays)
    Xsc = _build_xsc(x)                                   # (NCORES, NBLK, K, F, BLKN)

    nc = _build_program()
    in_maps = [{"xsc": Xsc[c], "w": W} for c in range(NCORES)]
    res = run_bass_kernel_spmd(nc, in_maps, core_ids=list(range(NCORES)))
    LAST_RESULT = res

    osc = np.stack([r["osc"] for r in res.results], axis=0)   # (NCORES,NS,128,CTOT)
    osc = osc.astype(np.float32)
    o = osc.reshape(NCORES, NS, 8, E, BPC, NCOLB)             # core, s, d, e, b, n
    o = o.transpose(0, 4, 5, 1, 2, 3)                          # core, b, n, s, d, e
    o = np.ascontiguousarray(o).reshape(B, NCOLB * Q, E)[:, :S, :]
    o = np.ascontiguousarray(o)
    o[:, S - 1, :] = 0.0                                   # reference zero-pads last column
    return o


# revision 9
# speedup vs baseline: 1.2333x; 1.1302x over previous
"""Trainium2 Bass kernel for nn_EventTemplateBank (batched 1-D template-bank conv).

Math: score[b,t,e] = sum_{f,l} delayed[e,f,l] * x[b, t+40-l, f] / (L*F),
with delayed = delay-shifted templates (zero fill) and x zero-padded.

Device formulation (per core, data-parallel over batch):
  Feature-packed Toeplitz: contraction partitions hold (feature, tap-chunk)
  pairs, K = 6*21 = 126, so one matmul contracts all 6 features over a
  21-tap window. Each rhs column covers Q=24 output positions; the 103-tap
  span (24+79) is accumulated over NCH=5 chunks. Outputs (d in [0,24),
  e in [0,16)) form NM=3 M-tiles of 128.
    X[(f,c), j, col=n] = x[b, 24n + 21j + c - 39, f]      (bf16, host im2col)
    W[(f,c), j, m, (dd,e)] = delayed[e, f, 8m+dd+79-21j-c] / 480
    ps[m][(dd,e), n] += sum_j W[:, j, m].T @ X[:, j, n-block]
  163,920 column-passes/core (vs 196,704 for the single-feature window) =
  68.3 us PE floor; 22.6 MB DMA/core ~= 71 us: balanced rooflines.
  PSUM f32 -> bf16 evac; host upcasts and re-permutes (t = 24n + 8m + dd).
"""

import numpy as np
import ml_dtypes

import concourse.mybir as mybir
from concourse import bacc
from concourse.bass_utils import run_bass_kernel_spmd
from concourse.tile import TileContext

BF16 = ml_dtypes.bfloat16

# Problem shapes (hardcoded per contract)
B, S, F = 64, 32768, 6
E, L = 16, 80
MAX_DELAY = 10

NCORES = 8
BPC = B // NCORES          # batches per core
Q = 24                     # output positions per rhs column
C = 21                     # taps per feature per chunk
NCH = 5                    # accumulation chunks (cover 24+79 = 103 <= 105 taps)
KP = F * C                 # 126 contraction partitions
NM = 3                     # M-tiles: (dd in [0,8)) x (e in [0,16)) per tile
PADL = 39                  # chunk sample index = 24n + 21j + c - 39
NCOLB = (S + Q - 1) // Q   # 1366 columns per batch
CTOT = BPC * NCOLB         # 10928 columns per core
# Small blocks first so the PE starts after ~0.25MB of input and ramps its
# p-state on cheap blocks; small remainder last for a short drain chain.
BLOCKS = [128, 128, 256] + [512] * 20 + [176]
assert sum(BLOCKS) == CTOT
NBLK = len(BLOCKS)
BOFF = [sum(BLOCKS[:i]) for i in range(NBLK)]
N_JOUTER = 3               # leading blocks run j-outer (gate on per-j W pieces)

LAST_RESULT = None         # BassKernelResults of the most recent run (for profiling)


def _build_weights(templates: np.ndarray, onset_delays: np.ndarray) -> np.ndarray:
    """W[(f,c), j, m, 16dd+e] = delayed[e, f, 8m+dd+79-21j-c] / (L*F)."""
    d = np.round(np.clip(onset_delays, -MAX_DELAY, MAX_DELAY)).astype(np.int64)
    idx = np.arange(L)
    src = idx[None, None, :] - d[:, :, None]                 # (E,F,L)
    valid = (src >= 0) & (src < L)
    delayed = np.take_along_axis(templates, np.clip(src, 0, L - 1), axis=2)
    delayed = np.where(valid, delayed, 0.0).astype(np.float32) / float(L * F)

    f_i = np.arange(F)[:, None, None, None, None]
    c_i = np.arange(C)[None, :, None, None, None]
    j_i = np.arange(NCH)[None, None, :, None, None]
    dd_i = np.arange(8)[None, None, None, :, None]
    m_i = np.arange(NM)[None, None, None, None, :]
    l = (8 * m_i + dd_i) + 79 - 21 * j_i - c_i               # (F,C,NCH,8,NM)
    ok = (l >= 0) & (l < L)
    g = delayed[:, f_i, np.clip(l, 0, L - 1)]                # (E,F,C,NCH,8,NM)
    g = np.where(ok[None], g, 0.0)
    # -> [(f,c), j, m, dd, e]
    W = g.transpose(1, 2, 3, 5, 4, 0).reshape(KP, NCH, NM, 8 * E)
    return np.ascontiguousarray(W).astype(BF16)


def _build_xsc(x: np.ndarray) -> np.ndarray:
    """Xsc[core, (f,c), :] = block-major concat of [NCH, n_blk] chunk rows:
    chunk j of column col = 1366*b_local + n reads x[b, 24n + 21j + c - 39, f]."""
    need = Q * (NCOLB - 1) + 21 * (NCH - 1) + C
    xpad = np.zeros((B, PADL + need, F), dtype=np.float32)
    xpad[:, PADL:PADL + S, :] = x
    sb, st, sf = xpad.strides
    # V[b, (f,c), j, n] = xpad[b, 24n + 21j + c, f]
    V = np.lib.stride_tricks.as_strided(
        xpad, shape=(B, F, C, NCH, NCOLB), strides=(sb, sf, st, 21 * st, Q * st)
    )
    V16 = V.astype(BF16).reshape(B, KP, NCH, NCOLB)
    Xc = np.empty((NCORES, KP, NCH, CTOT), dtype=BF16)
    for b in range(B):
        core, i = divmod(b, BPC)
        Xc[core, :, :, i * NCOLB:(i + 1) * NCOLB] = V16[b]
    out = np.empty((NCORES, KP, NCH * CTOT), dtype=BF16)
    for off, n in zip(BOFF, BLOCKS):
        out[:, :, NCH * off:NCH * (off + n)] = (
            Xc[:, :, :, off:off + n].reshape(NCORES, KP, NCH * n)
        )
    return np.ascontiguousarray(out)


def _build_program():
    f32 = mybir.dt.float32
    bf16 = mybir.dt.bfloat16
    nc = bacc.Bacc("TRN2", target_bir_lowering=False, debug=False)
    xsc = nc.dram_tensor("xsc", [KP, NCH * CTOT], bf16, kind="ExternalInput")
    w = nc.dram_tensor("w", [KP, NCH, NM, 128], bf16, kind="ExternalInput")
    osc = nc.dram_tensor("osc", [128, NM * CTOT], bf16, kind="ExternalOutput")

    with TileContext(nc) as tc:
        with (
            tc.tile_pool(name="wp", bufs=1) as wp,
            tc.tile_pool(name="xp", bufs=6) as xp,
            tc.tile_pool(name="pp", bufs=8, space="PSUM") as pp,
            tc.tile_pool(name="op", bufs=6) as op,
        ):
            wt = wp.tile([KP, NCH * NM * 128], bf16)     # [(f,c), (j, m, col)]
            wr = w.rearrange("k j m n -> k (j m n)")
            xtiles = {}

            def issue_w(j):
                sl = slice(j * NM * 128, (j + 1) * NM * 128)
                nc.sync.dma_start(out=wt[:, sl], in_=wr[:, sl])

            def issue_x(blk):
                off, n = BOFF[blk], BLOCKS[blk]
                xt = xp.tile([KP, NCH * n], bf16, tag="xt", name=f"xt_{blk}")
                nc.sync.dma_start(out=xt, in_=xsc[:, NCH * off:NCH * (off + n)])
                xtiles[blk] = xt

            def wslice(j, m):
                return wt[:, (j * NM + m) * 128:(j * NM + m + 1) * 128]

            # DMA order: first matmul gates on W(j0)+X0 (~0.25MB); later
            # pieces and x blocks stream in behind it.
            issue_w(0); issue_x(0)
            issue_w(1); issue_x(1)
            issue_w(2); issue_x(2)
            issue_w(3); issue_w(4)

            for blk in range(NBLK):
                off, n = BOFF[blk], BLOCKS[blk]
                if blk + 3 < NBLK:
                    issue_x(blk + 3)
                xt = xtiles.pop(blk)
                pss = [
                    pp.tile([128, n], f32, tag="ps", name=f"ps_{blk}_{m}")
                    for m in range(NM)
                ]
                ot = op.tile([128, NM * n], bf16, tag="ot", name=f"ot_{blk}")

                def evac(m, n=n, pss=pss, ot=ot):
                    nc.vector.tensor_copy(out=ot[:, m * n:(m + 1) * n], in_=pss[m])

                if blk < N_JOUTER:
                    # j-outer: each arriving W(j) piece feeds all 3 M-tiles.
                    for j in range(NCH):
                        for m in range(NM):
                            nc.tensor.matmul(
                                pss[m],
                                wslice(j, m),
                                xt[:, j * n:(j + 1) * n],
                                start=(j == 0),
                                stop=(j == NCH - 1),
                                skip_group_check=True,
                            )
                    for m in range(NM):
                        evac(m)
                else:
                    # m-outer: M-tiles complete one after another, so PSUM
                    # evacuation staggers across the block.
                    for m in range(NM):
                        for j in range(NCH):
                            nc.tensor.matmul(
                                pss[m],
                                wslice(j, m),
                                xt[:, j * n:(j + 1) * n],
                                start=(j == 0),
                                stop=(j == NCH - 1),
                            )
                        evac(m)
                nc.sync.dma_start(
                    out=osc[:, NM * off:NM * (off + n)], in_=ot
                )
    nc.compile()   # bacc passes: split multi-waits (HW allows 1 wait/inst), DCE, reg alloc
    return nc


def kernel(x: np.ndarray, templates: np.ndarray, onset_delays: np.ndarray) -> np.ndarray:
    global LAST_RESULT
    x = np.ascontiguousarray(x, dtype=np.float32)
    templates = np.asarray(templates, dtype=np.float32)
    onset_delays = np.asarray(onset_delays, dtype=np.float32)

    W = _build_weights(templates, onset_delays)
    Xsc = _build_xsc(x)                                   # (NCORES, KP, NCH*CTOT)

    nc = _build_program()
    in_maps = [{"xsc": Xsc[c], "w": W} for c in range(NCORES)]
    res = run_bass_kernel_spmd(nc, in_maps, core_ids=list(range(NCORES)))
    LAST_RESULT = res

    osc = np.stack([r["osc"] for r in res.results], axis=0)   # (NCORES,128,NM*CTOT)
    osc = osc.astype(np.float32)
    O = np.empty((NCORES, 128, NM, CTOT), dtype=np.float32)
    for off, n in zip(BOFF, BLOCKS):
        O[:, :, :, off:off + n] = (
            osc[:, :, NM * off:NM * (off + n)].reshape(NCORES, 128, NM, n)
        )
    o = O.reshape(NCORES, 8, E, NM, BPC, NCOLB)           # c, dd, e, m, b, n
    o = o.transpose(0, 4, 5, 3, 1, 2)                      # c, b, n, m, dd, e
    o = np.ascontiguousarray(o).reshape(B, NCOLB * Q, E)[:, :S, :]
    o = np.ascontiguousarray(o)
    o[:, S - 1, :] = 0.0                                   # reference zero-pads last column
    return o
